# revision 16
# baseline (speedup 1.0000x reference)
"""Trainium2 Bass kernel for CompressiveMemory (dpfp linear-attention memory).

Shapes (hardcoded): q,k,v [4,16,4096,128] f32, memory [1,16,256,128] f32,
z_norm [1,16,1,256] f32.  Output [4,16,4096,128] f32.

Sharding: the 64 (b,h) pairs are fully independent (memory/z_norm broadcast
over batch, updated per-pair). 8 pairs per core, zero communication.

Math (per pair, u = un-normalized dpfp(k), p = un-normalized dpfp(q),
sigma[s] = sum_j u[s,j], r[s] = sum_j u[s,j]^2):
  v_prev[s,:] = (u[s,:] @ M) / (u[s,:] . (z+EPS))
  w[s,:]      = (v[s,:] - v_prev[s,:]) / sigma[s]
  Mnew        = M + u^T w
  znew[j]     = z[j] + sum_s u[s,j]/sigma[s] - z[j]*sum_s u[s,j]^2/r[s]
  out[s,:]    = (p[s,:] @ Mnew) / (p[s,:] . (znew+EPS))
All matmuls bf16 with fp32 PSUM accumulation.
"""

import sys

for _p in ("/opt/trn_rl_repo", "/root/.axon_site/_ro/trn_rl_repo"):
    if _p not in sys.path:
        sys.path.insert(0, _p)

import numpy as np
import ml_dtypes

BF16 = ml_dtypes.bfloat16

B, H, S, D = 4, 16, 4096, 128
DK = 256
EPS = 1e-8
NCORES = 8
PPC = (B * H) // NCORES  # pairs per core = 8
NT = S // 128  # 32 S-tiles per pair
SLAB = 8  # S-tiles per elementwise slab
NSLAB = NT // SLAB  # 4
GRP = 4  # S-tiles per retrieve-psum group
CHUNK = 16  # S-tiles per DMA chunk
NCHUNK = NT // CHUNK  # 2

_CACHE = {}


def _build_program():
    import concourse.bass as bass
    import concourse.mybir as mybir
    import concourse.tile as tile
    from concourse import bacc
    from contextlib import ExitStack

    f32 = mybir.dt.float32
    bf16 = mybir.dt.bfloat16
    Alu = mybir.AluOpType
    Act = mybir.ActivationFunctionType

    nc = bacc.Bacc()
    kin = nc.declare_dram_parameter("kin", [PPC, S, D], f32, isOutput=False)
    vin = nc.declare_dram_parameter("vin", [PPC, S, D], f32, isOutput=False)
    qin = nc.declare_dram_parameter("qin", [PPC, S, D], f32, isOutput=False)
    # [M_chunk | -(z+EPS) | ones] per Dk-chunk, bf16
    rhsk = nc.declare_dram_parameter("rhsk", [PPC, 2, 128, 130], bf16, isOutput=False)
    mfp = nc.declare_dram_parameter("mfp", [PPC, 2, 128, 128], f32, isOutput=False)
    # zz[...,0] = z+EPS, zz[...,1] = -z
    zz = nc.declare_dram_parameter("zz", [PPC, 2, 128, 2], f32, isOutput=False)
    ident = nc.declare_dram_parameter("ident", [128, 128], bf16, isOutput=False)
    outd = nc.declare_dram_parameter("out", [PPC, S, D], f32, isOutput=True)

    with ExitStack() as ctx:
        tc = ctx.enter_context(tile.TileContext(nc))
        # --- SBUF pools ---
        p_in = ctx.enter_context(tc.tile_pool(name="p_in", bufs=3))
        p_v = ctx.enter_context(tc.tile_pool(name="p_v", bufs=3))
        p_ut = ctx.enter_context(tc.tile_pool(name="p_ut", bufs=4))
        p_t = ctx.enter_context(tc.tile_pool(name="p_t", bufs=2))
        p_u = ctx.enter_context(tc.tile_pool(name="p_u", bufs=3))
        p_u2 = ctx.enter_context(tc.tile_pool(name="p_u2", bufs=2))
        p_w = ctx.enter_context(tc.tile_pool(name="p_w", bufs=3))
        p_nd = ctx.enter_context(tc.tile_pool(name="p_nd", bufs=3))
        p_sc = ctx.enter_context(tc.tile_pool(name="p_sc", bufs=4))
        p_out = ctx.enter_context(tc.tile_pool(name="p_out", bufs=3))
        p_one = ctx.enter_context(tc.tile_pool(name="p_one", bufs=1))
        p_rq = ctx.enter_context(tc.tile_pool(name="p_rq", bufs=2))

        # --- PSUM pools ---
        ps_tr = ctx.enter_context(tc.tile_pool(name="ps_tr", bufs=2, space="PSUM"))
        ps_rk = ctx.enter_context(tc.tile_pool(name="ps_rk", bufs=2, space="PSUM"))
        ps_up = ctx.enter_context(tc.tile_pool(name="ps_up", bufs=1, space="PSUM"))
        ps_c2 = ctx.enter_context(tc.tile_pool(name="ps_c2", bufs=1, space="PSUM"))

        id_sb = p_one.tile([128, 128], bf16, tag="ident")
        nc.gpsimd.dma_start(out=id_sb, in_=ident[:, :])

        def feat_slab(src_chunks, s, t_sl, u_sl, ut_list):
            """Normal-layout dpfp features for slab s (SLAB S-tiles), plus
            PE-transposed copies (one [128,GRP,2,128] tile per 4-tile group)."""
            c, off = divmod(s * SLAB, CHUNK)
            src = src_chunks[c]
            sl = src[:, off:off + SLAB, :]
            nc.scalar.activation(out=t_sl[:, :, 0:128], in_=sl, func=Act.Relu)
            nc.scalar.activation(out=t_sl[:, :, 128:256], in_=sl, func=Act.Relu,
                                 scale=-1.0)
            nc.vector.tensor_mul(out=u_sl[:, :, 1:256], in0=t_sl[:, :, 1:256],
                                 in1=t_sl[:, :, 0:255])
            nc.vector.tensor_mul(out=u_sl[:, :, 0:1], in0=t_sl[:, :, 0:1],
                                 in1=t_sl[:, :, 255:256])
            for hs in range(SLAB // GRP):
                psht = ps_tr.tile([128, GRP, 2, 128], bf16, tag="pst",
                                  name=f"psht_{s}_{hs}")
                for j4 in range(GRP):
                    j = hs * GRP + j4
                    nc.tensor.transpose(out=psht[:, j4, 0, :],
                                        in_=u_sl[:, j, 0:128], identity=id_sb)
                    nc.tensor.transpose(out=psht[:, j4, 1, :],
                                        in_=u_sl[:, j, 128:256], identity=id_sb)
                ut = p_ut.tile([128, GRP, 2, 128], bf16, tag="uT",
                               name=f"ut_{s}_{hs}")
                nc.scalar.activation(out=ut.rearrange("p a b c -> p (a b c)"),
                                     in_=psht.rearrange("p a b c -> p (a b c)"),
                                     func=Act.Copy)
                ut_list.append(ut)

        for pair in range(PPC):
            k_d = kin[pair].rearrange("(n p) d -> p n d", p=128)
            v_d = vin[pair].rearrange("(n p) d -> p n d", p=128)
            q_d = qin[pair].rearrange("(n p) d -> p n d", p=128)
            o_d = outd[pair].rearrange("(n p) d -> p n d", p=128)

            kbf, vbf = [], []
            for c in range(NCHUNK):
                kt = p_in.tile([128, CHUNK, 128], bf16, tag="kq_in")
                nc.gpsimd.dma_start(out=kt, in_=k_d[:, c * CHUNK:(c + 1) * CHUNK, :])
                kbf.append(kt)
                vt = p_v.tile([128, CHUNK, 128], bf16, tag="v_in")
                nc.gpsimd.dma_start(out=vt, in_=v_d[:, c * CHUNK:(c + 1) * CHUNK, :])
                vbf.append(vt)

            rk_sb = p_one.tile([128, 2, 130], bf16, tag=f"rhsk{pair % 2}")
            nc.gpsimd.dma_start(out=rk_sb, in_=rhsk[pair].rearrange("c p n -> p c n"))
            mf_sb = p_one.tile([128, 2, 128], f32, tag=f"mfp{pair % 2}")
            nc.gpsimd.dma_start(out=mf_sb, in_=mfp[pair].rearrange("c p n -> p c n"))
            zz_sb = p_one.tile([128, 2, 2], f32, tag=f"zz{pair % 2}")
            nc.gpsimd.dma_start(out=zz_sb, in_=zz[pair].rearrange("c p n -> p c n"))

            r_all = p_sc.tile([128, NT], f32, tag="r_all")
            recr_bf = p_sc.tile([128, NT], bf16, tag="recr")
            ps_upd = ps_up.tile([128, 2, 256], f32, tag="upd")
            ps_cs2 = ps_c2.tile([1, 256], f32, tag="cs2")

            # =============== K-side: features + retrieve + update ===============
            for s in range(NSLAB):
                ut_list = []
                t_sl = p_t.tile([128, SLAB, 256], bf16, tag="t_n")
                u_sl = p_u.tile([128, SLAB, 256], bf16, tag="u_n")
                feat_slab(kbf, s, t_sl, u_sl, ut_list)

                u2_sl = p_u2.tile([128, SLAB, 256], bf16, tag="u2")
                for j in range(SLAB):
                    i = s * SLAB + j
                    nc.vector.scalar_tensor_tensor(
                        out=u2_sl[:, j, :], in0=u_sl[:, j, :], scalar=1.0,
                        in1=u_sl[:, j, :], op0=Alu.mult, op1=Alu.mult,
                        accum_out=r_all[:, i:i + 1])
                rec_t = p_sc.tile([128, SLAB], f32, tag="rec_t")
                nc.vector.reciprocal(out=rec_t, in_=r_all[:, s * SLAB:(s + 1) * SLAB])
                nc.vector.tensor_copy(out=recr_bf[:, s * SLAB:(s + 1) * SLAB],
                                      in_=rec_t)

                for g in range(s * SLAB // GRP, (s + 1) * SLAB // GRP):
                    ut = ut_list[g - s * SLAB // GRP]
                    prk = ps_rk.tile([128, GRP, 256], f32, tag="retr")
                    for jj in range(GRP):
                        nc.tensor.matmul(out=prk[:, jj, 0:130], lhsT=ut[:, jj, 0, :],
                                         rhs=rk_sb[:, 0, :], start=True, stop=False)
                        nc.tensor.matmul(out=prk[:, jj, 0:130], lhsT=ut[:, jj, 1, :],
                                         rhs=rk_sb[:, 1, :], start=False, stop=True)
                    nd = p_nd.tile([128, GRP, 128], bf16, tag="nd")
                    nc.scalar.activation(out=nd, in_=prk[:, :, 0:128], func=Act.Copy)
                    ds = p_sc.tile([128, GRP, 2], f32, tag="ds")
                    nc.scalar.activation(out=ds, in_=prk[:, :, 128:130], func=Act.Copy)
                    recd = p_sc.tile([128, GRP], f32, tag="recd")
                    recs = p_sc.tile([128, GRP], f32, tag="recs")
                    nc.vector.reciprocal(out=recd, in_=ds[:, :, 0])
                    nc.vector.reciprocal(out=recs, in_=ds[:, :, 1])
                    w_sl = p_w.tile([128, GRP, 129], bf16, tag="w")
                    nc.vector.tensor_copy(out=w_sl[:, :, 128], in_=recs)
                    for jj in range(GRP):
                        i = g * GRP + jj
                        j = i - s * SLAB
                        c, off = divmod(i, CHUNK)
                        vu = p_sc.tile([128, 128], bf16, tag="vu")
                        # vupd = v + numer*recd  (recd = -1/den)
                        nc.vector.scalar_tensor_tensor(
                            out=vu, in0=nd[:, jj, :], scalar=recd[:, jj:jj + 1],
                            in1=vbf[c][:, off, :], op0=Alu.mult, op1=Alu.add)
                        nc.vector.tensor_scalar_mul(out=w_sl[:, jj, 0:128], in0=vu,
                                                    scalar1=recs[:, jj:jj + 1])
                        nc.tensor.matmul(out=ps_upd[:, 0, 0:129],
                                         lhsT=u_sl[:, j, 0:128], rhs=w_sl[:, jj, :],
                                         start=(i == 0), stop=(i == NT - 1))
                        nc.tensor.matmul(out=ps_upd[:, 1, 0:129],
                                         lhsT=u_sl[:, j, 128:256], rhs=w_sl[:, jj, :],
                                         start=(i == 0), stop=(i == NT - 1))
                        nc.tensor.matmul(out=ps_cs2[:, :], lhsT=recr_bf[:, i:i + 1],
                                         rhs=u2_sl[:, j, :],
                                         start=(i == 0), stop=(i == NT - 1))

            # =============== state finalize -> rhs_q ===============
            cs2row = p_sc.tile([1, 256], bf16, tag="cs2row")
            nc.scalar.activation(out=cs2row, in_=ps_cs2[:, :], func=Act.Copy)
            ps_csc = ps_tr.tile([128, 2, 2], bf16, tag="pst")
            for c in range(2):
                nc.tensor.transpose(out=ps_csc[:, c, 0:1],
                                    in_=cs2row[0:1, c * 128:(c + 1) * 128],
                                    identity=id_sb[0:1, 0:1])
            rq_sb = p_rq.tile([128, 2, 129], bf16, tag="rq")
            for c in range(2):
                nc.vector.tensor_add(out=rq_sb[:, c, 0:128],
                                     in0=ps_upd[:, c, 0:128], in1=mf_sb[:, c, :])
                t2 = p_sc.tile([128, 1], f32, tag=f"t2_{c}")
                nc.vector.tensor_add(out=t2, in0=ps_upd[:, c, 128:129],
                                     in1=zz_sb[:, c, 0:1])
                nc.vector.scalar_tensor_tensor(
                    out=rq_sb[:, c, 128:129], in0=ps_csc[:, c, 0:1],
                    scalar=zz_sb[:, c, 1:2], in1=t2, op0=Alu.mult, op1=Alu.add)

            # =============== Q-side ===============
            qbf = []
            for c in range(NCHUNK):
                qt = p_in.tile([128, CHUNK, 128], bf16, tag="kq_in")
                nc.gpsimd.dma_start(out=qt, in_=q_d[:, c * CHUNK:(c + 1) * CHUNK, :])
                qbf.append(qt)
            out_sb = [p_out.tile([128, CHUNK, 128], f32, tag="out_sb",
                                 name=f"out_sb_{pair}_{c}")
                      for c in range(NCHUNK)]
            for s in range(NSLAB):
                pt_list = []
                t_sl = p_t.tile([128, SLAB, 256], bf16, tag="t_n")
                u_sl = p_u.tile([128, SLAB, 256], bf16, tag="u_n")
                feat_slab(qbf, s, t_sl, u_sl, pt_list)
                for g in range(s * SLAB // GRP, (s + 1) * SLAB // GRP):
                    pt = pt_list[g - s * SLAB // GRP]
                    prq = ps_rk.tile([128, GRP, 256], f32, tag="retr")
                    for jj in range(GRP):
                        nc.tensor.matmul(out=prq[:, jj, 0:129], lhsT=pt[:, jj, 0, :],
                                         rhs=rq_sb[:, 0, :], start=True, stop=False)
                        nc.tensor.matmul(out=prq[:, jj, 0:129], lhsT=pt[:, jj, 1, :],
                                         rhs=rq_sb[:, 1, :], start=False, stop=True)
                    dq = p_sc.tile([128, GRP], f32, tag="dq")
                    nc.scalar.activation(out=dq, in_=prq[:, :, 128], func=Act.Copy)
                    rdq = p_sc.tile([128, GRP], f32, tag="rdq")
                    nc.vector.reciprocal(out=rdq, in_=dq)
                    for jj in range(GRP):
                        i = g * GRP + jj
                        c, off = divmod(i, CHUNK)
                        nc.scalar.activation(out=out_sb[c][:, off, :],
                                             in_=prq[:, jj, 0:128], func=Act.Copy,
                                             scale=rdq[:, jj:jj + 1])
                    if (g + 1) * GRP % CHUNK == 0:
                        c = (g + 1) * GRP // CHUNK - 1
                        nc.gpsimd.dma_start(
                            out=o_d[:, c * CHUNK:(c + 1) * CHUNK, :], in_=out_sb[c])
    nc.compile()
    return nc


def _get_program():
    if "nc" not in _CACHE:
        _CACHE["nc"] = _build_program()
    return _CACHE["nc"]


def _host_prep(q, k, v, memory, z_norm):
    q = np.ascontiguousarray(q, dtype=np.float32).reshape(B * H, S, D)
    k = np.ascontiguousarray(k, dtype=np.float32).reshape(B * H, S, D)
    v = np.ascontiguousarray(v, dtype=np.float32).reshape(B * H, S, D)
    memory = np.asarray(memory, dtype=np.float32).reshape(H, DK, D)
    z_norm = np.asarray(z_norm, dtype=np.float32).reshape(H, DK)

    in_maps = []
    for core in range(NCORES):
        lo, hi = core * PPC, (core + 1) * PPC
        rhsk = np.zeros((PPC, 2, 128, 130), dtype=BF16)
        mfp = np.zeros((PPC, 2, 128, 128), dtype=np.float32)
        zz = np.zeros((PPC, 2, 128, 2), dtype=np.float32)
        for pi, g in enumerate(range(lo, hi)):
            h = g % H
            M = memory[h]
            z = z_norm[h]
            for c in range(2):
                Mc = M[c * 128:(c + 1) * 128, :]
                zc = z[c * 128:(c + 1) * 128]
                rhsk[pi, c, :, 0:128] = Mc.astype(BF16)
                rhsk[pi, c, :, 128] = (-(zc + EPS)).astype(BF16)
                rhsk[pi, c, :, 129] = BF16(1.0)
                mfp[pi, c] = Mc
                zz[pi, c, :, 0] = zc + EPS
                zz[pi, c, :, 1] = -zc
        in_maps.append({
            "kin": np.ascontiguousarray(k[lo:hi]),
            "vin": np.ascontiguousarray(v[lo:hi]),
            "qin": np.ascontiguousarray(q[lo:hi]),
            "rhsk": rhsk,
            "mfp": mfp,
            "zz": zz,
            "ident": np.eye(128, dtype=BF16),
        })
    return in_maps


def run_on_cores(q, k, v, memory, z_norm, **kw):
    from concourse.bass_utils import run_bass_kernel_spmd

    nc = _get_program()
    in_maps = _host_prep(q, k, v, memory, z_norm)
    res = run_bass_kernel_spmd(nc, in_maps, core_ids=list(range(NCORES)), **kw)
    outs = np.stack([r["out"] for r in res.results])  # [8, PPC, S, D]
    return outs.reshape(B, H, S, D).astype(np.float32), res


def kernel(q, k, v, memory, z_norm):
    out, _ = run_on_cores(q, k, v, memory, z_norm)
    return out
